# revision 9
# baseline (speedup 1.0000x reference)
"""Distance-loss kernel for Trainium2 (8 NeuronCores, SPMD data-parallel).

loss = sum_{b,c,h} || output[b,c,h,:] - target[b,c,h,:] + eps ||_2

Strategy: flatten both (16,8,512,512) f32 inputs to rows of W=512
(B*C*H = 65536 rows), shard rows contiguously across 8 cores (8192
rows/core).  Each core streams its 2 x 16 MiB in [128, 8, 512] tiles
(2 MiB per DMA), computes d = (x + eps) - y on the vector engine,
squares+row-reduces on the scalar (ACT) engine, then sqrt+reduces the
per-row norms to a [128,1] per-partition partial.  Host sums the 8x128
partials.  Memory-bound: per-core roofline ~= 32 MiB / 358 GB/s ~= 93 us.
"""

import numpy as np

import concourse.tile as tile
from concourse import bacc, bass_utils, mybir

EPS = 1e-6
N_CORES = 8
B, C, H, W = 16, 8, 512, 512
ROWS = B * C * H  # 65536 total rows of length W
ROWS_PER_CORE = ROWS // N_CORES  # 8192
P = 128  # SBUF partitions
R = 8    # rows packed per partition line (16 KiB contiguous per partition)


def build_bass(rows_per_core: int = ROWS_PER_CORE, bufs: int = 3, loops: int = 1):
    """Build the per-core SPMD Bass program.

    loops > 1 repeats the streaming body (same data) for timing-by-delta;
    the output is unchanged (the repeats are idempotent).
    """
    tiles = rows_per_core // (P * R)
    assert tiles * P * R == rows_per_core

    nc = bacc.Bacc("TRN2", target_bir_lowering=False, debug=False)
    x = nc.dram_tensor("x", [rows_per_core, W], mybir.dt.float32, kind="ExternalInput").ap()
    y = nc.dram_tensor("y", [rows_per_core, W], mybir.dt.float32, kind="ExternalInput").ap()
    out = nc.dram_tensor("out", [P, 1], mybir.dt.float32, kind="ExternalOutput").ap()

    xv = x.rearrange("(t p r) w -> t p r w", t=tiles, p=P, r=R)
    yv = y.rearrange("(t p r) w -> t p r w", t=tiles, p=P, r=R)

    with tile.TileContext(nc) as tc:
        with (
            tc.tile_pool(name="xp", bufs=bufs) as xp,
            tc.tile_pool(name="yp", bufs=bufs) as yp,
            tc.tile_pool(name="dp", bufs=2) as dp,
            tc.tile_pool(name="sq", bufs=2) as sqp,
            tc.tile_pool(name="st", bufs=1) as stp,
        ):
            # per-row sums of squares: one column per (tile, packed-row)
            rowsq = stp.tile([P, tiles * R], mybir.dt.float32)
            for t in range(tiles * loops):
                t = t % tiles
                xt = xp.tile([P, R, W], mybir.dt.float32)
                nc.sync.dma_start(xt[:], xv[t])
                yt = yp.tile([P, R, W], mybir.dt.float32)
                nc.sync.dma_start(yt[:], yv[t])

                d = dp.tile([P, R, W], mybir.dt.float32)
                # d = (x + eps) - y   (one DVE pass over the 2 MiB tile)
                nc.vector.scalar_tensor_tensor(
                    out=d[:],
                    in0=xt[:],
                    scalar=EPS,
                    in1=yt[:],
                    op0=mybir.AluOpType.add,
                    op1=mybir.AluOpType.subtract,
                )
                # per row: sum of squares via ACT Square + free-dim accumulate
                for j in range(R):
                    sq = sqp.tile([P, W], mybir.dt.float32)
                    nc.scalar.activation(
                        out=sq[:],
                        in_=d[:, j, :],
                        func=mybir.ActivationFunctionType.Square,
                        accum_out=rowsq[:, t * R + j : t * R + j + 1],
                    )

            # row_norm = sqrt(rowsq); per-partition partial = sum(row_norm)
            norms = stp.tile([P, tiles * R], mybir.dt.float32)
            rowsum = stp.tile([P, 1], mybir.dt.float32)
            nc.scalar.activation(
                out=norms[:],
                in_=rowsq[:],
                func=mybir.ActivationFunctionType.Sqrt,
                accum_out=rowsum[:],
            )
            nc.sync.dma_start(out[:], rowsum[:])
    nc.compile()
    return nc


def build_bass_looped(loops: int):
    return build_bass(loops=loops)


_NC_CACHE = {}


def kernel(output: np.ndarray, target: np.ndarray) -> np.ndarray:
    assert output.shape == (B, C, H, W) and target.shape == (B, C, H, W)
    if "nc" not in _NC_CACHE:
        _NC_CACHE["nc"] = build_bass()
    nc = _NC_CACHE["nc"]

    X = np.ascontiguousarray(output, dtype=np.float32).reshape(N_CORES, ROWS_PER_CORE, W)
    Y = np.ascontiguousarray(target, dtype=np.float32).reshape(N_CORES, ROWS_PER_CORE, W)
    in_maps = [{"x": X[k], "y": Y[k]} for k in range(N_CORES)]
    res = bass_utils.run_bass_kernel_spmd(nc, in_maps, core_ids=list(range(N_CORES)))
    total = 0.0
    for m in res.results:
        total += float(m["out"].astype(np.float64).sum())
    return np.asarray(total, dtype=np.float32)
